# revision 6
# baseline (speedup 1.0000x reference)
"""CIN (xDeepFM compressed interaction network) forward kernel for 8 Trainium2
NeuronCores, data-parallel over the batch.

Math (per layer i; h = current feature maps of width Fk; F0=39, D=16, S=128):
    out[b,s,d] = relu(sum_{f,g} x[b,f,d] * h[b,g,d] * W[f,g,s] + bias[s])
    split_half: h_next = out[:, :64]; direct_i = out[:, 64:] (last layer: all)
    result[b, :] = sum_d concat(direct_0, direct_1, direct_2)   # (B, 256)

Per-core mapping (batch shard of 256 rows): columns c = b*16 + d, C = 4096.
    Z^T[(f,g), c] = xT[f, c] * hT[g, c]          (DVE tensor_tensor, bf16)
    psum[s, c]   += W_chunk[(f,g), s].T @ Z^T    (PE, K-chunks accumulated)
The xT[f] broadcast tiles (xb) are pre-expanded on the HOST into DRAM (xE
tensors) so the device reads them with plain contiguous DMA (~350 GB/s)
instead of stride-0 broadcast descriptors (~48 GB/s). Layers 1 and 2 share
the same xb tiles (read once per slab, kept in SBUF through both layers).
"""
import sys

sys.path.insert(0, '/opt/trn_rl_repo')
sys.path.insert(0, '/root/problem')

import numpy as np

import tile_patch

tile_patch.apply()

import concourse.bass as bass
import concourse.mybir as mybir
import concourse.tile as tile

B, F0, D, S = 2048, 39, 16, 128
NCORES = 8
BSH = B // NCORES            # 256 batch rows per core
C = BSH * D                  # 4096 columns per core
H = S // 2                   # 64
CCHUNK = 512                 # matmul N / one PSUM bank
TTW = 2048                   # tensor_tensor slab width
NTT = C // TTW               # 2 slabs
K0, NCH0 = 117, 13           # layer 0 chunking (3 f-blocks x 39 g)
NCH12 = 20                   # layer 1/2 chunking (2 f-blocks x 64 g)

_cached = {}


def build_kernel():
    nc = bass.Bass("TRN2")
    dt = mybir.dt
    xS = nc.dram_tensor("xS", [K0, C], dt.bfloat16, kind="ExternalInput")
    xE0 = nc.dram_tensor("xE0", [NTT * NCH0 * K0, TTW], dt.bfloat16,
                         kind="ExternalInput")
    xE12 = nc.dram_tensor("xE12", [NTT * NCH12 * 128, TTW], dt.bfloat16,
                          kind="ExternalInput")
    w0 = nc.dram_tensor("w0", [NCH0 * K0, S], dt.bfloat16, kind="ExternalInput")
    w1 = nc.dram_tensor("w1", [NCH12 * 128, S], dt.bfloat16, kind="ExternalInput")
    w2 = nc.dram_tensor("w2", [NCH12 * 128, S], dt.bfloat16, kind="ExternalInput")
    bias = nc.dram_tensor("bias", [3 * S], dt.float32, kind="ExternalInput")
    out = nc.dram_tensor("out", [BSH, 2 * S], dt.float32, kind="ExternalOutput")

    with tile.TileContext(nc) as tc:
        with tc.tile_pool(name="const", bufs=1) as cpool, \
             tc.tile_pool(name="hpool", bufs=1) as hpool, \
             tc.tile_pool(name="xb0p", bufs=1) as xb0p, \
             tc.tile_pool(name="xb12p", bufs=1) as xb12p, \
             tc.tile_pool(name="zzp", bufs=6) as zzp, \
             tc.tile_pool(name="evp", bufs=4) as evp, \
             tc.tile_pool(name="psum", bufs=1, space="PSUM") as psp, \
             tc.tile_pool(name="pst", bufs=2, space="PSUM") as pst:

            from concourse.masks import make_identity
            tw0 = cpool.tile([K0, NCH0, S], dt.bfloat16)
            nc.sync.dma_start(out=tw0, in_=w0.rearrange("(k p) s -> p k s", p=K0))
            tw1 = cpool.tile([128, NCH12, S], dt.bfloat16)
            nc.sync.dma_start(out=tw1, in_=w1.rearrange("(k p) s -> p k s", p=128))
            tw2 = cpool.tile([128, NCH12, S], dt.bfloat16)
            nc.sync.dma_start(out=tw2, in_=w2.rearrange("(k p) s -> p k s", p=128))
            tbias = cpool.tile([S, 3], dt.float32)
            nc.sync.dma_start(out=tbias,
                              in_=bass.AP(tensor=bias, offset=0, ap=[[1, S], [S, 3]]))
            ident = cpool.tile([128, 128], dt.float32)
            make_identity(nc, ident)

            hh0 = hpool.tile([K0, C], dt.bfloat16)
            nc.sync.dma_start(out=hh0, in_=xS[:, :])
            hh1 = hpool.tile([128, C], dt.bfloat16)
            hh2 = hpool.tile([128, C], dt.bfloat16)
            dres = [hpool.tile([128, BSH], dt.float32, name=f"dres{i}")
                    for i in range(3)]

            def do_layer(li, tw, hh, kp, nch, tt, xbt, hnext):
                """One layer's work for slab tt; xbt[k] = xb SBUF tiles."""
                c0 = tt * TTW
                pss = [psp.tile([128, CCHUNK], dt.float32, name=f"ps{cc}")
                       for cc in range(TTW // CCHUNK)]
                for k in range(nch):
                    z = zzp.tile([128, TTW], dt.bfloat16, name="zz")
                    nc.vector.tensor_mul(z[0:kp, :], hh[:, c0:c0 + TTW],
                                         xbt[k][0:kp, :])
                    for cc in range(TTW // CCHUNK):
                        nc.tensor.matmul(
                            pss[cc], tw[:, k, :],
                            z[0:kp, cc * CCHUNK:(cc + 1) * CCHUNK],
                            start=(k == 0), stop=(k == nch - 1))
                bcol = tbias[:, li:li + 1]
                for cc in range(TTW // CCHUNK):
                    cs = c0 + cc * CCHUNK
                    ps = pss[cc]
                    rd = evp.tile([128, CCHUNK], dt.bfloat16, name="rd")
                    if hnext is not None:
                        nc.scalar.activation(
                            out=hnext[0:H, cs:cs + CCHUNK], in_=ps[0:H, :],
                            func=mybir.ActivationFunctionType.Relu,
                            bias=bcol[0:H, :], scale=1.0)
                        nc.scalar.activation(
                            out=rd[H:128, :], in_=ps[H:128, :],
                            func=mybir.ActivationFunctionType.Relu,
                            bias=bcol[H:128, :], scale=1.0)
                        nc.vector.tensor_reduce(
                            out=dres[li][H:128, cs // D:(cs + CCHUNK) // D],
                            in_=rd[H:128, :].rearrange("p (b d) -> p b d", d=D),
                            op=mybir.AluOpType.add, axis=mybir.AxisListType.X)
                    else:
                        nc.scalar.activation(
                            out=rd, in_=ps,
                            func=mybir.ActivationFunctionType.Relu,
                            bias=bcol, scale=1.0)
                        nc.vector.tensor_reduce(
                            out=dres[li][:, cs // D:(cs + CCHUNK) // D],
                            in_=rd.rearrange("p (b d) -> p b d", d=D),
                            op=mybir.AluOpType.add, axis=mybir.AxisListType.X)
                if hnext is not None:
                    nc.sync.dma_start(out=hnext[H:128, c0:c0 + TTW],
                                      in_=hnext[0:H, c0:c0 + TTW])

            # ---- layer 0 (own xb tiles, rotating) ----
            for tt in range(NTT):
                xbt = []
                for k in range(NCH0):
                    xb = xb0p.tile([K0, TTW], dt.bfloat16, name=f"xb0_{k % 6}")
                    r0 = (tt * NCH0 + k) * K0
                    eng = nc.sync if k % 2 == 0 else nc.scalar
                    eng.dma_start(out=xb, in_=xE0[r0:r0 + K0, :])
                    xbt.append(xb)
                do_layer(0, tw0, hh0, K0, NCH0, tt, xbt, hh1)

            # ---- layers 1+2 interleaved per slab (shared xb tiles) ----
            for tt in range(NTT):
                xbt = []
                for k in range(NCH12):
                    xb = xb12p.tile([128, TTW], dt.bfloat16, name=f"xb12_{k}")
                    r0 = (tt * NCH12 + k) * 128
                    eng = nc.sync if k % 2 == 0 else nc.scalar
                    eng.dma_start(out=xb, in_=xE12[r0:r0 + 128, :])
                    xbt.append(xb)
                do_layer(1, tw1, hh1, 128, NCH12, tt, xbt, hh2)
                do_layer(2, tw2, hh2, 128, NCH12, tt, xbt, None)

            # ---- final: transpose direct sums to [b, s_cat] ----
            for bb in range(BSH // 128):
                bsl = slice(bb * 128, bb * 128 + 128)
                pt = pst.tile([128, 256], dt.float32, name="pt")
                nc.tensor.transpose(pt[:, 0:64], dres[0][H:128, bsl],
                                    ident[H:128, H:128])
                nc.tensor.transpose(pt[:, 64:128], dres[1][H:128, bsl],
                                    ident[H:128, H:128])
                nc.tensor.transpose(pt[:, 128:256], dres[2][:, bsl], ident)
                ot = evp.tile([128, 256], dt.float32, name="ot")
                nc.scalar.copy(ot, pt)
                nc.sync.dma_start(out=out[bb * 128:(bb + 1) * 128, :], in_=ot)
    return nc


# host-side expansion index maps
_fidx0 = np.array([3 * k + p // 39 for k in range(NCH0) for p in range(K0)])
_fidx12 = np.array([min(2 * k + (p >> 6), F0 - 1)
                    for k in range(NCH12) for p in range(128)])


def _prep_core(xTbf, w0, w1, w2, bias):
    xS = np.ascontiguousarray(np.tile(xTbf, (3, 1)))          # [117, C]
    e0 = xTbf[_fidx0]                                          # [1521, C]
    xE0 = np.ascontiguousarray(
        e0.reshape(NCH0 * K0, NTT, TTW).transpose(1, 0, 2)).reshape(-1, TTW)
    e12 = xTbf[_fidx12]                                        # [2560, C]
    xE12 = np.ascontiguousarray(
        e12.reshape(NCH12 * 128, NTT, TTW).transpose(1, 0, 2)).reshape(-1, TTW)
    return {"xS": xS, "xE0": xE0, "xE12": xE12,
            "w0": w0, "w1": w1, "w2": w2, "bias": bias}


def _prep_inputs(x, W0, W1, W2, b0, b1, b2):
    import ml_dtypes
    bf16 = ml_dtypes.bfloat16
    x = np.asarray(x, np.float32)

    def padw(w):
        w = np.asarray(w, np.float32)
        return np.concatenate(
            [w, np.zeros((NCH12 * 128 - w.shape[0], S), np.float32)], axis=0
        ).astype(bf16)

    w0 = np.asarray(W0, np.float32).astype(bf16)
    w1, w2 = padw(np.asarray(W1)), padw(np.asarray(W2))
    bias = np.concatenate([np.asarray(b0, np.float32), np.asarray(b1, np.float32),
                           np.asarray(b2, np.float32)]).astype(np.float32)
    in_maps = []
    for c in range(NCORES):
        xs = x[c * BSH:(c + 1) * BSH]
        xTc = np.ascontiguousarray(xs.transpose(1, 0, 2).reshape(F0, C)).astype(bf16)
        in_maps.append(_prep_core(xTc, w0, w1, w2, bias))
    return in_maps


def _get_runner(nc, in_maps):
    """jit-once runner over the axon PJRT tunnel (multi-core shard_map)."""
    import jax
    from jax.sharding import Mesh, PartitionSpec
    from jax.experimental.shard_map import shard_map
    from concourse import bass2jax

    bass2jax.install_neuronx_cc_hook()
    partition_name = (nc.partition_id_tensor.name if nc.partition_id_tensor
                      else None)
    in_names, out_names, out_avals = [], [], []
    for alloc in nc.m.functions[0].allocations:
        if not isinstance(alloc, mybir.MemoryLocationSet):
            continue
        name = alloc.memorylocations[0].name
        if alloc.kind == "ExternalInput":
            if name != partition_name:
                in_names.append(name)
        elif alloc.kind == "ExternalOutput":
            out_names.append(name)
            out_avals.append(jax.core.ShapedArray(
                tuple(alloc.tensor_shape), mybir.dt.np(alloc.dtype)))
    n_params = len(in_names)
    all_in_names = in_names + out_names
    if partition_name is not None:
        all_in_names.append(partition_name)

    def _body(*args):
        operands = list(args)
        if partition_name is not None:
            operands.append(bass2jax.partition_id_tensor())
        outs = bass2jax._bass_exec_p.bind(
            *operands, out_avals=tuple(out_avals), in_names=tuple(all_in_names),
            out_names=tuple(out_names), lowering_input_output_aliases=(),
            sim_require_finite=True, sim_require_nnan=True, nc=nc)
        return tuple(outs)

    devices = jax.devices()[:NCORES]
    mesh = Mesh(np.asarray(devices), ("core",))
    n_outs = len(out_names)
    sharded = jax.jit(
        shard_map(_body, mesh=mesh,
                  in_specs=(PartitionSpec("core"),) * (n_params + n_outs),
                  out_specs=(PartitionSpec("core"),) * n_outs,
                  check_rep=False),
        keep_unused=True)
    zero_outs = [np.zeros((NCORES * a.shape[0], *a.shape[1:]), a.dtype)
                 for a in out_avals]
    zeros_dev = [jax.device_put(z) for z in zero_outs]

    def run(in_maps):
        concat_in = [
            np.concatenate([np.asarray(in_maps[c][name]) for c in range(NCORES)],
                           axis=0)
            for name in in_names
        ]
        outs = sharded(*concat_in, *zeros_dev)
        jax.block_until_ready(outs)
        return [
            {name: np.asarray(outs[i]).reshape(NCORES, *out_avals[i].shape)[c]
             for i, name in enumerate(out_names)}
            for c in range(NCORES)
        ]

    return run


def kernel(x, W0, W1, W2, b0, b1, b2):
    in_maps = _prep_inputs(x, W0, W1, W2, b0, b1, b2)
    if "run" not in _cached:
        nc = build_kernel()
        _cached["run"] = _get_runner(nc, in_maps)
    results = _cached["run"](in_maps)
    return np.concatenate([results[c]["out"] for c in range(NCORES)], axis=0)


# revision 7
# speedup vs baseline: 8.4907x; 8.4907x over previous
"""CIN (xDeepFM compressed interaction network) forward kernel for 8 Trainium2
NeuronCores, data-parallel over the batch.

Math (per layer i; h = current feature maps of width Fk; F0=39, D=16, S=128):
    out[b,s,d] = relu(sum_{f,g} x[b,f,d] * h[b,g,d] * W[f,g,s] + bias[s])
    split_half: h_next = out[:, :64]; direct_i = out[:, 64:] (last layer: all)
    result[b, :] = sum_d concat(direct_0, direct_1, direct_2)   # (B, 256)

Per-core mapping (batch shard of 256 rows): columns c = b*16 + d, C = 4096.
    Z^T[(f,g), c] = xT[f, c] * hT[g, c]          (DVE tensor_tensor, bf16)
    psum[s, c]   += W_chunk[(f,g), s].T @ Z^T    (PE, K-chunks accumulated)
The xT[f] broadcast tiles (xb) are pre-expanded on the HOST into DRAM (xE
tensors) so the device reads them with plain contiguous DMA (~350 GB/s)
instead of stride-0 broadcast descriptors (~48 GB/s). Layers 1 and 2 share
the same xb tiles (read once per slab, kept in SBUF through both layers).
"""
import sys

sys.path.insert(0, '/opt/trn_rl_repo')
sys.path.insert(0, '/root/problem')

import numpy as np

import tile_patch

tile_patch.apply()

import concourse.bass as bass
import concourse.mybir as mybir
import concourse.tile as tile

B, F0, D, S = 2048, 39, 16, 128
NCORES = 8
BSH = B // NCORES            # 256 batch rows per core
C = BSH * D                  # 4096 columns per core
H = S // 2                   # 64
CCHUNK = 512                 # matmul N / one PSUM bank
TTW = 2048                   # tensor_tensor slab width
NTT = C // TTW               # 2 slabs
K0, NCH0 = 117, 13           # layer 0 chunking (3 f-blocks x 39 g)
NCH12 = 20                   # layer 1/2 chunking (2 f-blocks x 64 g)

_cached = {}


def build_kernel():
    nc = bass.Bass("TRN2")
    dt = mybir.dt
    xT = nc.dram_tensor("xT", [F0, C], dt.bfloat16, kind="ExternalInput")
    w0 = nc.dram_tensor("w0", [NCH0 * K0, S], dt.bfloat16, kind="ExternalInput")
    w1 = nc.dram_tensor("w1", [NCH12 * 128, S], dt.bfloat16, kind="ExternalInput")
    w2 = nc.dram_tensor("w2", [NCH12 * 128, S], dt.bfloat16, kind="ExternalInput")
    bias = nc.dram_tensor("bias", [3 * S], dt.float32, kind="ExternalInput")
    out = nc.dram_tensor("out", [BSH, 2 * S], dt.float32, kind="ExternalOutput")

    with tile.TileContext(nc) as tc:
        with tc.tile_pool(name="const", bufs=1) as cpool, \
             tc.tile_pool(name="hpool", bufs=1) as hpool, \
             tc.tile_pool(name="xb0p", bufs=1) as xb0p, \
             tc.tile_pool(name="xb12p", bufs=1) as xb12p, \
             tc.tile_pool(name="zzp", bufs=6) as zzp, \
             tc.tile_pool(name="evp", bufs=4) as evp, \
             tc.tile_pool(name="psum", bufs=1, space="PSUM") as psp, \
             tc.tile_pool(name="pst", bufs=2, space="PSUM") as pst:

            from concourse.masks import make_identity
            tw0 = cpool.tile([K0, NCH0, S], dt.bfloat16)
            nc.sync.dma_start(out=tw0, in_=w0.rearrange("(k p) s -> p k s", p=K0))
            tw1 = cpool.tile([128, NCH12, S], dt.bfloat16)
            nc.sync.dma_start(out=tw1, in_=w1.rearrange("(k p) s -> p k s", p=128))
            tw2 = cpool.tile([128, NCH12, S], dt.bfloat16)
            nc.sync.dma_start(out=tw2, in_=w2.rearrange("(k p) s -> p k s", p=128))
            tbias = cpool.tile([S, 3], dt.float32)
            nc.sync.dma_start(out=tbias,
                              in_=bass.AP(tensor=bias, offset=0, ap=[[1, S], [S, 3]]))
            ident = cpool.tile([128, 128], dt.float32)
            make_identity(nc, ident)

            hh0 = hpool.tile([K0, C], dt.bfloat16)
            for j in range(3):
                nc.sync.dma_start(out=hh0[j * F0:(j + 1) * F0, :], in_=xT[:, :])
            hh1 = hpool.tile([128, C], dt.bfloat16)
            hh2 = hpool.tile([128, C], dt.bfloat16)
            dres = [hpool.tile([128, BSH], dt.float32, name=f"dres{i}")
                    for i in range(3)]

            def do_layer(li, tw, hh, kp, nch, tt, xbt, hnext):
                """One layer's work for slab tt; xbt[k] = xb SBUF tiles."""
                c0 = tt * TTW
                pss = [psp.tile([128, CCHUNK], dt.float32, name=f"ps{cc}")
                       for cc in range(TTW // CCHUNK)]
                for k in range(nch):
                    z = zzp.tile([128, TTW], dt.bfloat16, name="zz")
                    nc.vector.tensor_mul(z[0:kp, :], hh[:, c0:c0 + TTW],
                                         xbt[k][0:kp, :])
                    for cc in range(TTW // CCHUNK):
                        nc.tensor.matmul(
                            pss[cc], tw[:, k, :],
                            z[0:kp, cc * CCHUNK:(cc + 1) * CCHUNK],
                            start=(k == 0), stop=(k == nch - 1))
                bcol = tbias[:, li:li + 1]
                for cc in range(TTW // CCHUNK):
                    cs = c0 + cc * CCHUNK
                    ps = pss[cc]
                    rd = evp.tile([128, CCHUNK], dt.bfloat16, name="rd")
                    if hnext is not None:
                        nc.scalar.activation(
                            out=hnext[0:H, cs:cs + CCHUNK], in_=ps[0:H, :],
                            func=mybir.ActivationFunctionType.Relu,
                            bias=bcol[0:H, :], scale=1.0)
                        nc.scalar.activation(
                            out=rd[H:128, :], in_=ps[H:128, :],
                            func=mybir.ActivationFunctionType.Relu,
                            bias=bcol[H:128, :], scale=1.0)
                        nc.vector.tensor_reduce(
                            out=dres[li][H:128, cs // D:(cs + CCHUNK) // D],
                            in_=rd[H:128, :].rearrange("p (b d) -> p b d", d=D),
                            op=mybir.AluOpType.add, axis=mybir.AxisListType.X)
                    else:
                        nc.scalar.activation(
                            out=rd, in_=ps,
                            func=mybir.ActivationFunctionType.Relu,
                            bias=bcol, scale=1.0)
                        nc.vector.tensor_reduce(
                            out=dres[li][:, cs // D:(cs + CCHUNK) // D],
                            in_=rd.rearrange("p (b d) -> p b d", d=D),
                            op=mybir.AluOpType.add, axis=mybir.AxisListType.X)
                if hnext is not None:
                    nc.sync.dma_start(out=hnext[H:128, c0:c0 + TTW],
                                      in_=hnext[0:H, c0:c0 + TTW])

            # ---- layer 0 (own xb tiles, rotating) ----
            for tt in range(NTT):
                xbt = []
                c0 = tt * TTW
                for k in range(NCH0):
                    xb = xb0p.tile([K0, TTW], dt.bfloat16, name=f"xb0_{k % 6}")
                    for j in range(3):
                        f = 3 * k + j
                        eng = nc.sync if (k * 3 + j) % 2 == 0 else nc.scalar
                        eng.dma_start(
                            out=xb[j * F0:(j + 1) * F0, :],
                            in_=bass.AP(tensor=xT, offset=f * C + c0,
                                        ap=[[0, F0], [1, TTW]]))
                    xbt.append(xb)
                do_layer(0, tw0, hh0, K0, NCH0, tt, xbt, hh1)

            # ---- layers 1+2 interleaved per slab (shared xb tiles) ----
            for tt in range(NTT):
                xbt = []
                c0 = tt * TTW
                for k in range(NCH12):
                    xb = xb12p.tile([128, TTW], dt.bfloat16, name=f"xb12_{k}")
                    for j in range(2):
                        f = min(2 * k + j, F0 - 1)
                        eng = nc.sync if (k * 2 + j) % 2 == 0 else nc.scalar
                        eng.dma_start(
                            out=xb[j * H:(j + 1) * H, :],
                            in_=bass.AP(tensor=xT, offset=f * C + c0,
                                        ap=[[0, H], [1, TTW]]))
                    xbt.append(xb)
                do_layer(1, tw1, hh1, 128, NCH12, tt, xbt, hh2)
                do_layer(2, tw2, hh2, 128, NCH12, tt, xbt, None)

            # ---- final: transpose direct sums to [b, s_cat] ----
            for bb in range(BSH // 128):
                bsl = slice(bb * 128, bb * 128 + 128)
                pt = pst.tile([128, 256], dt.float32, name="pt")
                nc.tensor.transpose(pt[:, 0:64], dres[0][H:128, bsl],
                                    ident[H:128, H:128])
                nc.tensor.transpose(pt[:, 64:128], dres[1][H:128, bsl],
                                    ident[H:128, H:128])
                nc.tensor.transpose(pt[:, 128:256], dres[2][:, bsl], ident)
                ot = evp.tile([128, 256], dt.float32, name="ot")
                nc.scalar.copy(ot, pt)
                nc.sync.dma_start(out=out[bb * 128:(bb + 1) * 128, :], in_=ot)
    return nc


def _prep_inputs(x, W0, W1, W2, b0, b1, b2):
    import ml_dtypes
    bf16 = ml_dtypes.bfloat16
    x = np.asarray(x, np.float32)

    def padw(w):
        w = np.asarray(w, np.float32)
        return np.concatenate(
            [w, np.zeros((NCH12 * 128 - w.shape[0], S), np.float32)], axis=0
        ).astype(bf16)

    w0 = np.asarray(W0, np.float32).astype(bf16)
    w1, w2 = padw(np.asarray(W1)), padw(np.asarray(W2))
    bias = np.concatenate([np.asarray(b0, np.float32), np.asarray(b1, np.float32),
                           np.asarray(b2, np.float32)]).astype(np.float32)
    in_maps = []
    for c in range(NCORES):
        xs = x[c * BSH:(c + 1) * BSH]
        xTc = np.ascontiguousarray(xs.transpose(1, 0, 2).reshape(F0, C)).astype(bf16)
        in_maps.append({"xT": xTc, "w0": w0, "w1": w1, "w2": w2, "bias": bias})
    return in_maps


def _get_runner(nc, in_maps):
    """jit-once runner over the axon PJRT tunnel (multi-core shard_map)."""
    import jax
    from jax.sharding import Mesh, PartitionSpec
    from jax.experimental.shard_map import shard_map
    from concourse import bass2jax

    bass2jax.install_neuronx_cc_hook()
    partition_name = (nc.partition_id_tensor.name if nc.partition_id_tensor
                      else None)
    in_names, out_names, out_avals = [], [], []
    for alloc in nc.m.functions[0].allocations:
        if not isinstance(alloc, mybir.MemoryLocationSet):
            continue
        name = alloc.memorylocations[0].name
        if alloc.kind == "ExternalInput":
            if name != partition_name:
                in_names.append(name)
        elif alloc.kind == "ExternalOutput":
            out_names.append(name)
            out_avals.append(jax.core.ShapedArray(
                tuple(alloc.tensor_shape), mybir.dt.np(alloc.dtype)))
    n_params = len(in_names)
    all_in_names = in_names + out_names
    if partition_name is not None:
        all_in_names.append(partition_name)

    def _body(*args):
        operands = list(args)
        if partition_name is not None:
            operands.append(bass2jax.partition_id_tensor())
        outs = bass2jax._bass_exec_p.bind(
            *operands, out_avals=tuple(out_avals), in_names=tuple(all_in_names),
            out_names=tuple(out_names), lowering_input_output_aliases=(),
            sim_require_finite=True, sim_require_nnan=True, nc=nc)
        return tuple(outs)

    devices = jax.devices()[:NCORES]
    mesh = Mesh(np.asarray(devices), ("core",))
    n_outs = len(out_names)
    sharded = jax.jit(
        shard_map(_body, mesh=mesh,
                  in_specs=(PartitionSpec("core"),) * (n_params + n_outs),
                  out_specs=(PartitionSpec("core"),) * n_outs,
                  check_rep=False),
        keep_unused=True)
    zero_outs = [np.zeros((NCORES * a.shape[0], *a.shape[1:]), a.dtype)
                 for a in out_avals]
    zeros_dev = [jax.device_put(z) for z in zero_outs]

    def run(in_maps):
        concat_in = [
            np.concatenate([np.asarray(in_maps[c][name]) for c in range(NCORES)],
                           axis=0)
            for name in in_names
        ]
        outs = sharded(*concat_in, *zeros_dev)
        jax.block_until_ready(outs)
        return [
            {name: np.asarray(outs[i]).reshape(NCORES, *out_avals[i].shape)[c]
             for i, name in enumerate(out_names)}
            for c in range(NCORES)
        ]

    return run


def kernel(x, W0, W1, W2, b0, b1, b2):
    in_maps = _prep_inputs(x, W0, W1, W2, b0, b1, b2)
    if "run" not in _cached:
        nc = build_kernel()
        _cached["run"] = _get_runner(nc, in_maps)
    results = _cached["run"](in_maps)
    return np.concatenate([results[c]["out"] for c in range(NCORES)], axis=0)


# revision 9
# speedup vs baseline: 8.7599x; 1.0317x over previous
"""CIN (xDeepFM compressed interaction network) forward kernel for 8 Trainium2
NeuronCores, data-parallel over the batch.

Math (per layer i; h = current feature maps of width Fk; F0=39, D=16, S=128):
    out[b,s,d] = relu(sum_{f,g} x[b,f,d] * h[b,g,d] * W[f,g,s] + bias[s])
    split_half: h_next = out[:, :64]; direct_i = out[:, 64:] (last layer: all)
    result[b, :] = sum_d concat(direct_0, direct_1, direct_2)   # (B, 256)

Per-core mapping (batch shard of 256 rows): columns c = b*16 + d, C = 4096.
    Z^T[(f,g), c] = xT[f, c] * hT[g, c]          (DVE tensor_tensor, bf16)
    psum[s, c]   += W_chunk[(f,g), s].T @ Z^T    (PE, K-chunks accumulated)
The xT[f] broadcast tiles (xb) are pre-expanded on the HOST into DRAM (xE
tensors) so the device reads them with plain contiguous DMA (~350 GB/s)
instead of stride-0 broadcast descriptors (~48 GB/s). Layers 1 and 2 share
the same xb tiles (read once per slab, kept in SBUF through both layers).
"""
import sys

sys.path.insert(0, '/opt/trn_rl_repo')
sys.path.insert(0, '/root/problem')

import numpy as np

import tile_patch

tile_patch.apply()

import concourse.bass as bass
import concourse.mybir as mybir
import concourse.tile as tile

B, F0, D, S = 2048, 39, 16, 128
NCORES = 8
BSH = B // NCORES            # 256 batch rows per core
C = BSH * D                  # 4096 columns per core
H = S // 2                   # 64
CCHUNK = 512                 # matmul N / one PSUM bank
TTW = 2048                   # tensor_tensor slab width
NTT = C // TTW               # 2 slabs
NCH12 = 20                   # layer 1/2 chunking (2 f-blocks x 64 g)

# Symmetric layer 0: W0s[f,g] = W0[f,g] + W0[g,f] (f<g), W0[f,f] on the diag;
# rows (f, g>=f) packed f-major into chunks of whole f-blocks, <=128 rows each.
def _l0_chunks():
    chunks, cur, cur_len = [], [], 0
    for f in range(F0):
        blk = F0 - f                       # rows g = f..38
        if cur_len + blk > 128:
            chunks.append(cur)
            cur, cur_len = [], 0
        cur.append(f)
        cur_len += blk
    chunks.append(cur)
    return chunks

L0CH = _l0_chunks()                        # list of f-lists per chunk
L0LEN = [sum(F0 - f for f in ch) for ch in L0CH]
L0OFF = np.concatenate([[0], np.cumsum(L0LEN)])[:-1]
NCH0 = len(L0CH)
K0MAX = max(L0LEN)

_cached = {}


def build_kernel(reps=1):
    nc = bass.Bass("TRN2")
    dt = mybir.dt
    xT = nc.dram_tensor("xT", [F0, C], dt.bfloat16, kind="ExternalInput")
    w0 = nc.dram_tensor("w0", [780, S], dt.bfloat16, kind="ExternalInput")
    w1 = nc.dram_tensor("w1", [NCH12 * 128, S], dt.bfloat16, kind="ExternalInput")
    w2 = nc.dram_tensor("w2", [NCH12 * 128, S], dt.bfloat16, kind="ExternalInput")
    bias = nc.dram_tensor("bias", [3 * S], dt.float32, kind="ExternalInput")
    out = nc.dram_tensor("out", [BSH, 2 * S], dt.float32, kind="ExternalOutput")

    with tile.TileContext(nc) as tc:
        with tc.tile_pool(name="const", bufs=1) as cpool, \
             tc.tile_pool(name="hpool", bufs=1) as hpool, \
             tc.tile_pool(name="xb0p", bufs=1) as xb0p, \
             tc.tile_pool(name="xb12p", bufs=1) as xb12p, \
             tc.tile_pool(name="zzp", bufs=6) as zzp, \
             tc.tile_pool(name="evp", bufs=4) as evp, \
             tc.tile_pool(name="psum", bufs=1, space="PSUM") as psp, \
             tc.tile_pool(name="pst", bufs=2, space="PSUM") as pst:

            from concourse.masks import make_identity
            tw0 = cpool.tile([K0MAX, NCH0, S], dt.bfloat16)
            for k in range(NCH0):
                nc.sync.dma_start(out=tw0[0:L0LEN[k], k, :],
                                  in_=w0[int(L0OFF[k]):int(L0OFF[k]) + L0LEN[k], :])
            tw1 = cpool.tile([128, NCH12, S], dt.bfloat16)
            nc.sync.dma_start(out=tw1, in_=w1.rearrange("(k p) s -> p k s", p=128))
            tw2 = cpool.tile([128, NCH12, S], dt.bfloat16)
            nc.sync.dma_start(out=tw2, in_=w2.rearrange("(k p) s -> p k s", p=128))
            tbias = cpool.tile([S, 3], dt.float32)
            nc.sync.dma_start(out=tbias,
                              in_=bass.AP(tensor=bias, offset=0, ap=[[1, S], [S, 3]]))
            ident = cpool.tile([128, 128], dt.float32)
            make_identity(nc, ident)

            hh1 = hpool.tile([128, C], dt.bfloat16)
            hh2 = hpool.tile([128, C], dt.bfloat16)
            dres = [hpool.tile([128, BSH], dt.float32, name=f"dres{i}")
                    for i in range(3)]

            def do_layer(li, tw, hh, kp, nch, tt, xbt, hnext, het=None,
                         klens=None):
                """One layer's work for slab tt; xbt[k] = xb SBUF tiles.
                If het is given (layer 0), in0 per chunk is het[k] instead of
                a slice of the persistent hh tile; klens gives per-chunk K."""
                c0 = tt * TTW
                pss = [psp.tile([128, CCHUNK], dt.float32, name=f"ps{cc}")
                       for cc in range(TTW // CCHUNK)]
                for k in range(nch):
                    kp_k = klens[k] if klens is not None else kp
                    z = zzp.tile([128, TTW], dt.bfloat16, name="zz")
                    in0 = het[k][0:kp_k, :] if het is not None \
                        else hh[:, c0:c0 + TTW]
                    nc.vector.tensor_mul(z[0:kp_k, :], in0, xbt[k][0:kp_k, :])
                    for cc in range(TTW // CCHUNK):
                        nc.tensor.matmul(
                            pss[cc], tw[0:kp_k, k, :],
                            z[0:kp_k, cc * CCHUNK:(cc + 1) * CCHUNK],
                            start=(k == 0), stop=(k == nch - 1))
                bcol = tbias[:, li:li + 1]
                for cc in range(TTW // CCHUNK):
                    cs = c0 + cc * CCHUNK
                    ps = pss[cc]
                    rd = evp.tile([128, CCHUNK], dt.bfloat16, name="rd")
                    if hnext is not None:
                        nc.scalar.activation(
                            out=hnext[0:H, cs:cs + CCHUNK], in_=ps[0:H, :],
                            func=mybir.ActivationFunctionType.Relu,
                            bias=bcol[0:H, :], scale=1.0)
                        nc.scalar.activation(
                            out=rd[H:128, :], in_=ps[H:128, :],
                            func=mybir.ActivationFunctionType.Relu,
                            bias=bcol[H:128, :], scale=1.0)
                        nc.vector.tensor_reduce(
                            out=dres[li][H:128, cs // D:(cs + CCHUNK) // D],
                            in_=rd[H:128, :].rearrange("p (b d) -> p b d", d=D),
                            op=mybir.AluOpType.add, axis=mybir.AxisListType.X)
                    else:
                        nc.scalar.activation(
                            out=rd, in_=ps,
                            func=mybir.ActivationFunctionType.Relu,
                            bias=bcol, scale=1.0)
                        nc.vector.tensor_reduce(
                            out=dres[li][:, cs // D:(cs + CCHUNK) // D],
                            in_=rd.rearrange("p (b d) -> p b d", d=D),
                            op=mybir.AluOpType.add, axis=mybir.AxisListType.X)
                if hnext is not None:
                    nc.sync.dma_start(out=hnext[H:128, c0:c0 + TTW],
                                      in_=hnext[0:H, c0:c0 + TTW])

            # ---- layer 0 (own xb tiles, rotating) ----
            for _rep in range(reps):
                for tt in range(NTT):
                    xbt, het = [], []
                    c0 = tt * TTW
                    dmac = 0
                    for k in range(NCH0):
                        xb = xb0p.tile([K0MAX, TTW], dt.bfloat16,
                                       name=f"xb0_{k % 4}")
                        he = xb0p.tile([K0MAX, TTW], dt.bfloat16,
                                       name=f"he0_{k % 4}")
                        r = 0
                        for f in L0CH[k]:
                            blk = F0 - f
                            eng = nc.sync if dmac % 2 == 0 else nc.scalar
                            # xb rows: xT[f] broadcast over blk partitions
                            eng.dma_start(
                                out=xb[r:r + blk, :],
                                in_=bass.AP(tensor=xT, offset=f * C + c0,
                                            ap=[[0, blk], [1, TTW]]))
                            # hE rows: xT[f..38] contiguous rows
                            eng2 = nc.scalar if dmac % 2 == 0 else nc.sync
                            eng2.dma_start(
                                out=he[r:r + blk, :],
                                in_=bass.AP(tensor=xT, offset=f * C + c0,
                                            ap=[[C, blk], [1, TTW]]))
                            r += blk
                            dmac += 1
                        xbt.append(xb)
                        het.append(he)
                    do_layer(0, tw0, None, None, NCH0, tt, xbt, hh1,
                             het=het, klens=L0LEN)

                # ---- layers 1+2 interleaved per slab (shared xb tiles) ----
                for tt in range(NTT):
                    xbt = []
                    c0 = tt * TTW
                    for k in range(NCH12):
                        xb = xb12p.tile([128, TTW], dt.bfloat16, name=f"xb12_{k}")
                        for j in range(2):
                            f = min(2 * k + j, F0 - 1)
                            eng = nc.sync if (k * 2 + j) % 2 == 0 else nc.scalar
                            eng.dma_start(
                                out=xb[j * H:(j + 1) * H, :],
                                in_=bass.AP(tensor=xT, offset=f * C + c0,
                                            ap=[[0, H], [1, TTW]]))
                        xbt.append(xb)
                    do_layer(1, tw1, hh1, 128, NCH12, tt, xbt, hh2)
                    do_layer(2, tw2, hh2, 128, NCH12, tt, xbt, None)

            # ---- final: transpose direct sums to [b, s_cat] ----
            for bb in range(BSH // 128):
                bsl = slice(bb * 128, bb * 128 + 128)
                pt = pst.tile([128, 256], dt.float32, name="pt")
                nc.tensor.transpose(pt[:, 0:64], dres[0][H:128, bsl],
                                    ident[H:128, H:128])
                nc.tensor.transpose(pt[:, 64:128], dres[1][H:128, bsl],
                                    ident[H:128, H:128])
                nc.tensor.transpose(pt[:, 128:256], dres[2][:, bsl], ident)
                ot = evp.tile([128, 256], dt.float32, name="ot")
                nc.scalar.copy(ot, pt)
                nc.sync.dma_start(out=out[bb * 128:(bb + 1) * 128, :], in_=ot)
    return nc


def _prep_inputs(x, W0, W1, W2, b0, b1, b2):
    import ml_dtypes
    bf16 = ml_dtypes.bfloat16
    x = np.asarray(x, np.float32)

    def padw(w):
        w = np.asarray(w, np.float32)
        return np.concatenate(
            [w, np.zeros((NCH12 * 128 - w.shape[0], S), np.float32)], axis=0
        ).astype(bf16)

    W0f = np.asarray(W0, np.float32).reshape(F0, F0, S)
    W0sym = W0f + W0f.transpose(1, 0, 2)
    rows = []
    for ch in L0CH:
        for f in ch:
            blk = W0sym[f, f:, :].copy()
            blk[0] = W0f[f, f, :]          # diagonal: plain W0[f,f]
            rows.append(blk)
    w0 = np.concatenate(rows, axis=0).astype(bf16)   # [780, S]
    w1, w2 = padw(np.asarray(W1)), padw(np.asarray(W2))
    bias = np.concatenate([np.asarray(b0, np.float32), np.asarray(b1, np.float32),
                           np.asarray(b2, np.float32)]).astype(np.float32)
    in_maps = []
    for c in range(NCORES):
        xs = x[c * BSH:(c + 1) * BSH]
        xTc = np.ascontiguousarray(xs.transpose(1, 0, 2).reshape(F0, C)).astype(bf16)
        in_maps.append({"xT": xTc, "w0": w0, "w1": w1, "w2": w2, "bias": bias})
    return in_maps


def _get_runner(nc, in_maps):
    """jit-once runner over the axon PJRT tunnel (multi-core shard_map)."""
    import jax
    from jax.sharding import Mesh, PartitionSpec
    from jax.experimental.shard_map import shard_map
    from concourse import bass2jax

    bass2jax.install_neuronx_cc_hook()
    partition_name = (nc.partition_id_tensor.name if nc.partition_id_tensor
                      else None)
    in_names, out_names, out_avals = [], [], []
    for alloc in nc.m.functions[0].allocations:
        if not isinstance(alloc, mybir.MemoryLocationSet):
            continue
        name = alloc.memorylocations[0].name
        if alloc.kind == "ExternalInput":
            if name != partition_name:
                in_names.append(name)
        elif alloc.kind == "ExternalOutput":
            out_names.append(name)
            out_avals.append(jax.core.ShapedArray(
                tuple(alloc.tensor_shape), mybir.dt.np(alloc.dtype)))
    n_params = len(in_names)
    all_in_names = in_names + out_names
    if partition_name is not None:
        all_in_names.append(partition_name)

    def _body(*args):
        operands = list(args)
        if partition_name is not None:
            operands.append(bass2jax.partition_id_tensor())
        outs = bass2jax._bass_exec_p.bind(
            *operands, out_avals=tuple(out_avals), in_names=tuple(all_in_names),
            out_names=tuple(out_names), lowering_input_output_aliases=(),
            sim_require_finite=True, sim_require_nnan=True, nc=nc)
        return tuple(outs)

    devices = jax.devices()[:NCORES]
    mesh = Mesh(np.asarray(devices), ("core",))
    n_outs = len(out_names)
    sharded = jax.jit(
        shard_map(_body, mesh=mesh,
                  in_specs=(PartitionSpec("core"),) * (n_params + n_outs),
                  out_specs=(PartitionSpec("core"),) * n_outs,
                  check_rep=False),
        keep_unused=True)
    zero_outs = [np.zeros((NCORES * a.shape[0], *a.shape[1:]), a.dtype)
                 for a in out_avals]
    zeros_dev = [jax.device_put(z) for z in zero_outs]

    def run(in_maps):
        concat_in = [
            np.concatenate([np.asarray(in_maps[c][name]) for c in range(NCORES)],
                           axis=0)
            for name in in_names
        ]
        outs = sharded(*concat_in, *zeros_dev)
        jax.block_until_ready(outs)
        return [
            {name: np.asarray(outs[i]).reshape(NCORES, *out_avals[i].shape)[c]
             for i, name in enumerate(out_names)}
            for c in range(NCORES)
        ]

    return run


def kernel(x, W0, W1, W2, b0, b1, b2):
    in_maps = _prep_inputs(x, W0, W1, W2, b0, b1, b2)
    if "run" not in _cached:
        nc = build_kernel()
        _cached["run"] = _get_runner(nc, in_maps)
    results = _cached["run"](in_maps)
    return np.concatenate([results[c]["out"] for c in range(NCORES)], axis=0)
